# revision 20
# baseline (speedup 1.0000x reference)
"""Trainium2 Bass kernel for BailingMoE linear (lightning) attention prefill.

Strategy: data-parallel over the 8192-token sequence across 8 NeuronCores
(1024 tokens / 4 lightning chunks per core). The only cross-core dependency is
the inter-chunk KV state scan; each core computes its local decayed KV sum
(U_c), a 1 MB AllGather shares all U_c, and each core folds in the states of
earlier cores via per-core data-driven weights (so the SPMD program is
identical on every core), then adds a per-chunk correction matmul.

All matmuls run in bf16 (f32 PSUM accumulation). Activations are kept
feature-major ([feature, token]) so the RMS norm reduction becomes a
ones-vector matmul and the output projection needs no transposes.
g_norm_weight is folded into Wd on the host.
"""

import sys

import numpy as np

for _p in ("/opt/trn_rl_repo",):
    if _p not in sys.path:
        sys.path.insert(0, _p)

import ml_dtypes  # noqa: E402
import concourse.bass as bass  # noqa: E402
import concourse.tile as tile  # noqa: E402
from concourse import bacc, mybir  # noqa: E402
from concourse.bass_utils import run_bass_kernel_spmd  # noqa: E402
from concourse.masks import make_identity  # noqa: E402

F32 = mybir.dt.float32
BF16 = mybir.dt.bfloat16
BF16_NP = ml_dtypes.bfloat16
AF = mybir.ActivationFunctionType
ALU = mybir.AluOpType

N, HID, H, D, BLOCK = 8192, 2048, 16, 128, 256
NCORES = 8
T = N // NCORES      # 1024 tokens per core
C = T // BLOCK       # 4 chunks per core
KI = HID // 128      # 16 contraction tiles
SC = D ** -0.5
EPS = 1e-5
THETA = 600000.0


def _slopes():
    start = 2.0 ** (-(2.0 ** (-(np.log2(H) - 3.0))))
    s = np.array([start * start**i for i in range(H)], dtype=np.float32)
    return s * np.float32(1.0 - 0.0 / (20 - 1) + 1e-5)


SLOPES = _slopes()
BD = np.exp(-SLOPES.astype(np.float64) * BLOCK).astype(np.float32)  # [H]


def build_program(dbg=False):
    nc = bacc.Bacc("TRN2", target_bir_lowering=False, debug=False,
                   num_devices=NCORES)

    # ---- per-core external inputs (host-preprocessed layouts) ----
    hT_d = nc.dram_tensor("hT", [128, KI, T], BF16, kind="ExternalInput")
    wqkg_d = nc.dram_tensor("wqkg", [48, 128, KI, 128], BF16, kind="ExternalInput")
    wv_d = nc.dram_tensor("wv", [2, KI, 128, 1024], BF16, kind="ExternalInput")
    wd_d = nc.dram_tensor("wd", [2, KI, 128, 1024], BF16, kind="ExternalInput")
    cos_d = nc.dram_tensor("cosT", [128, T], BF16, kind="ExternalInput")
    sin_d = nc.dram_tensor("sinT", [128, T], BF16, kind="ExternalInput")
    qdec_d = nc.dram_tensor("qdec", [H, BLOCK], BF16, kind="ExternalInput")
    kdec_d = nc.dram_tensor("kdec", [H, BLOCK], F32, kind="ExternalInput")
    diag_d = nc.dram_tensor("diagT", [H, 128, 2, BLOCK], BF16, kind="ExternalInput")
    scanw_d = nc.dram_tensor("scanw", [NCORES, H], F32, kind="ExternalInput")
    out_d = nc.dram_tensor("out", [T, HID], F32, kind="ExternalOutput")
    dbg_d = {}
    if dbg:
        for nm, shp in [("dbg_qT", [128, H, T]), ("dbg_kT", [128, H, T]),
                        ("dbg_v", [128, T // 128, HID]), ("dbg_sig", [128, H, T]),
                        ("dbg_attnT", [128, H, T]), ("dbg_U", [128, HID]),
                        ("dbg_S", [128, HID]), ("dbg_rms", [1, T])]:
            dbg_d[nm] = nc.dram_tensor(nm, shp, F32, kind="ExternalOutput")

    with tile.TileContext(nc) as tc:
        with (
            tc.tile_pool(name="persist", bufs=1) as persist,
            tc.tile_pool(name="kvs", bufs=1) as kvs_pool,
            tc.tile_pool(name="ha", bufs=1) as ha_pool,
            tc.tile_pool(name="wq", bufs=2) as wq_pool,
            tc.tile_pool(name="wvd", bufs=3) as wvd_pool,
            tc.tile_pool(name="braw", bufs=4) as braw_pool,
            tc.tile_pool(name="scratch", bufs=4) as scratch_pool,
            tc.tile_pool(name="kvb", bufs=4) as kvb_pool,
            tc.tile_pool(name="qk", bufs=4) as qk_pool,
            tc.tile_pool(name="diag", bufs=2) as diag_pool,
            tc.tile_pool(name="psum", bufs=8, space="PSUM") as psum_pool,
            tc.tile_pool(name="dram", bufs=1, space="DRAM") as dram_pool,
        ):
            # ---- persistent SBUF tiles ----
            hT = ha_pool.tile([128, KI, T], BF16, tag="ha")
            qT = persist.tile([128, H, T], BF16, tag="qT")
            kT = persist.tile([128, H, T], BF16, tag="kT")
            v_sb = persist.tile([128, T // 128, HID], BF16, tag="v")
            sig = persist.tile([128, H, T], BF16, tag="sig")
            cosT = persist.tile([128, T], BF16, tag="cos")
            sinT = persist.tile([128, T], BF16, tag="sin")
            kdec = persist.tile([128, H, 2], F32, tag="kdec")
            KV = kvs_pool.tile([128, HID], F32, tag="kvs")
            wbc = persist.tile([128, NCORES, H], F32, tag="wbc")
            ident = persist.tile([128, 128], BF16, tag="ident")
            ones = persist.tile([128, 1], BF16, tag="ones")
            eps_t = persist.tile([1, 1], F32, tag="eps")
            rmsb = persist.tile([128, T], BF16, tag="rmsb")

            ub = dram_pool.tile([128, HID], F32)
            ugb = dram_pool.tile([NCORES, 128, HID], F32, addr_space="Shared")
            rms_dram = dram_pool.tile([1, T], F32)

            make_identity(nc, ident[:, :])
            nc.vector.memset(eps_t[:, :], EPS)
            nc.vector.memset(ones[:, :], 1.0)

            nc.sync.dma_start(out=cosT[:, :], in_=cos_d.ap())
            nc.sync.dma_start(out=sinT[:, :], in_=sin_d.ap())
            # k_decay: token-major [128, 1] per (head, half-chunk)
            for h in range(H):
                for j in range(2):
                    nc.sync.dma_start(
                        out=kdec[:, h, j:j + 1],
                        in_=bass.AP(tensor=kdec_d, offset=h * BLOCK + j * 128,
                                    ap=[[1, 128], [1, 1]]),
                    )
            nc.sync.dma_start(
                out=wbc[:, :, :],
                in_=bass.AP(tensor=scanw_d, offset=0,
                            ap=[[0, 128], [H, NCORES], [1, H]]),
            )

            nc.sync.dma_start(out=hT[:, :, :], in_=hT_d.ap())

            # ================= P1a: q/k/gate projections (feature-major) ====
            # mi 0..15 -> q heads, 16..31 -> k heads, 32..47 -> gate tiles
            for mi in range(48):
                wt = wq_pool.tile([128, KI, 128], BF16, tag="wq")
                nc.sync.dma_start(out=wt[:, :, :], in_=wqkg_d[mi])
                ps = [psum_pool.tile([128, 512], F32, tag="ps", name=f"ps_{mi}_{i}") for i in range(2)]
                for ki in range(KI):
                    for ni in range(2):
                        nc.tensor.matmul(
                            ps[ni][:, :], wt[:, ki, :],
                            hT[:, ki, ni * 512:(ni + 1) * 512],
                            start=(ki == 0), stop=(ki == KI - 1),
                        )
                if mi < 32:  # q or k: RoPE epilogue
                    dstT = qT if mi < 16 else kT
                    h = mi % 16
                    for ni in range(2):
                        nsl = slice(ni * 512, (ni + 1) * 512)
                        raw = braw_pool.tile([128, 512], BF16, tag="braw")
                        nc.scalar.copy(out=raw[:, :], in_=ps[ni][:, :])
                        rh = braw_pool.tile([128, 512], BF16, tag="braw")
                        nc.scalar.activation(out=rh[0:64, :],
                                             in_=ps[ni][64:128, :],
                                             func=AF.Copy, scale=-1.0)
                        nc.scalar.copy(out=rh[64:128, :], in_=ps[ni][0:64, :])
                        tt = braw_pool.tile([128, 512], BF16, tag="braw")
                        dst = dstT[:, h, nsl]
                        nc.vector.tensor_tensor(tt[:, :], rh[:, :], sinT[:, nsl], ALU.mult)
                        nc.vector.tensor_tensor(dst, raw[:, :], cosT[:, nsl], ALU.mult)
                        nc.vector.tensor_tensor(dst, dst, tt[:, :], ALU.add)
                else:  # gate -> sigmoid
                    h = mi - 32
                    for ni in range(2):
                        nc.scalar.activation(
                            out=sig[:, h, ni * 512:(ni + 1) * 512],
                            in_=ps[ni][:, :], func=AF.Sigmoid,
                        )

            # ================= P1b: v projection (token-major) ==============
            for tq in range(2):
                for njp in range(2):
                    pv = [psum_pool.tile([128, 512], F32, tag="ps", name=f"pv_{tq}_{njp}_{i}")
                          for i in range(8)]
                    for ki in range(KI):
                        wvt = wvd_pool.tile([128, 1024], BF16, tag="wvd")
                        nc.sync.dma_start(out=wvt[:, :], in_=wv_d[njp, ki])
                        for tt in range(4):
                            t = tq * 4 + tt
                            for njc in range(2):
                                nc.tensor.matmul(
                                    pv[tt * 2 + njc][:, :],
                                    hT[:, ki, t * 128:(t + 1) * 128],
                                    wvt[:, njc * 512:(njc + 1) * 512],
                                    start=(ki == 0), stop=(ki == KI - 1),
                                )
                    for tt in range(4):
                        t = tq * 4 + tt
                        for njc in range(2):
                            csl = slice(njp * 1024 + njc * 512,
                                        njp * 1024 + njc * 512 + 512)
                            nc.scalar.copy(out=v_sb[:, t, csl],
                                           in_=pv[tt * 2 + njc][:, :])

            # ================= P2: A/KV scan + cross-chunk (local) ==========
            attnT = ha_pool.tile([128, H, T], BF16, tag="ha")
            for c in range(C):
                for h in range(H):
                    hs_ = slice(h * 128, (h + 1) * 128)
                    csl = slice(c * BLOCK, (c + 1) * BLOCK)
                    if c > 0:
                        kvb = kvb_pool.tile([128, 128], BF16, tag="kvb")
                        nc.scalar.copy(out=kvb[:, :], in_=KV[:, hs_])
                        qdc = qk_pool.tile([128, BLOCK], BF16, tag="qk",
                                           name=f"qdc2_{c}_{h}")
                        nc.sync.dma_start(
                            out=qdc[:, :],
                            in_=bass.AP(tensor=qdec_d, offset=h * BLOCK,
                                        ap=[[0, 128], [1, BLOCK]]))
                        qd_t = qk_pool.tile([128, BLOCK], BF16, tag="qk")
                        nc.vector.tensor_tensor(qd_t[:, :], qT[:, h, csl],
                                                qdc[:, :], ALU.mult)
                        op = psum_pool.tile([128, BLOCK], F32, tag="ps")
                        nc.tensor.matmul(op[:, :], kvb[:, :], qd_t[:, :],
                                         start=True, stop=True)
                        nc.vector.tensor_copy(attnT[:, h, csl], op[:, :])
                    kd = [None, None]
                    for j in range(2):
                        ktp = psum_pool.tile([128, 128], BF16, tag="ps",
                                             name=f"ktp_{c}_{h}_{j}")
                        nc.tensor.transpose(
                            ktp[:, :],
                            kT[:, h, c * BLOCK + j * 128: c * BLOCK + (j + 1) * 128],
                            ident[:, :])
                        kd[j] = kvb_pool.tile([128, 128], BF16, tag="kvb", name=f"kd_{c}_{h}_{j}")
                        nc.vector.tensor_scalar(kd[j][:, :], ktp[:, :],
                                                kdec[:, h, j:j + 1], None, ALU.mult)
                    ap_ = psum_pool.tile([128, 128], F32, tag="ps")
                    nc.tensor.matmul(ap_[:, :], kd[0][:, :],
                                     v_sb[:, 2 * c, hs_], start=True, stop=False)
                    nc.tensor.matmul(ap_[:, :], kd[1][:, :],
                                     v_sb[:, 2 * c + 1, hs_], start=False, stop=True)
                    if c == 0:
                        nc.vector.tensor_copy(KV[:, hs_], ap_[:, :])
                    else:
                        nc.vector.scalar_tensor_tensor(
                            out=KV[:, hs_], in0=KV[:, hs_], scalar=float(BD[h]),
                            in1=ap_[:, :], op0=ALU.mult, op1=ALU.add)

            # U_c = final KV -> AllGather
            nc.sync.dma_start(out=ub[:, :], in_=KV[:, :])
            nc.gpsimd.collective_compute(
                "AllGather", ALU.bypass,
                ins=[ub.opt()], outs=[ugb.opt()],
                replica_groups=[list(range(NCORES))],
            )

            # ================= P3: intra-chunk attention (overlaps AG) ======
            for c in range(C):
                for h in range(H):
                    hs_ = slice(h * 128, (h + 1) * 128)
                    csl = slice(c * BLOCK, (c + 1) * BLOCK)
                    dg = diag_pool.tile([128, 2, BLOCK], BF16, tag="diag")
                    nc.sync.dma_start(out=dg[:, :, :], in_=diag_d[h])
                    qs = [None, None]
                    for j in range(2):
                        qkp = psum_pool.tile([128, BLOCK], F32, tag="ps")
                        nc.tensor.matmul(
                            qkp[:, :],
                            kT[:, h, c * BLOCK + j * 128: c * BLOCK + (j + 1) * 128],
                            qT[:, h, csl], start=True, stop=True)
                        qs[j] = qk_pool.tile([128, BLOCK], BF16, tag="qk", name=f"qs_{c}_{h}_{j}")
                        nc.vector.tensor_tensor(qs[j][:, :], qkp[:, :],
                                                dg[:, j, :], ALU.mult)
                    o2 = psum_pool.tile([128, BLOCK], F32, tag="ps")
                    nc.tensor.matmul(o2[:, :], v_sb[:, 2 * c, hs_], qs[0][:, :],
                                     start=True, stop=False)
                    nc.tensor.matmul(o2[:, :], v_sb[:, 2 * c + 1, hs_], qs[1][:, :],
                                     start=False, stop=True)
                    if c == 0:
                        nc.vector.tensor_copy(attnT[:, h, csl], o2[:, :])
                    else:
                        nc.vector.tensor_tensor(attnT[:, h, csl], attnT[:, h, csl],
                                                o2[:, :], ALU.add)

            # ================= P4: combine gathered states ==================
            S = kvs_pool.tile([128, HID], F32, tag="kvs")
            for cp in range(NCORES):
                for g4 in range(4):
                    ug = scratch_pool.tile([128, 512], F32, tag="sc",
                                      name=f"ug_{cp}_{g4}")
                    nc.sync.dma_start(
                        out=ug[:, :],
                        in_=ugb[cp][:, g4 * 512:(g4 + 1) * 512])
                    for hh in range(4):
                        h = g4 * 4 + hh
                        hs_ = slice(h * 128, (h + 1) * 128)
                        us_ = slice(hh * 128, (hh + 1) * 128)
                        if cp == 0:
                            nc.vector.tensor_scalar(S[:, hs_], ug[:, us_],
                                                    wbc[:, cp, h:h + 1], None,
                                                    ALU.mult)
                        else:
                            nc.vector.scalar_tensor_tensor(
                                out=S[:, hs_], in0=ug[:, us_],
                                scalar=wbc[:, cp, h:h + 1], in1=S[:, hs_],
                                op0=ALU.mult, op1=ALU.add)

            # ================= P5: cross-core corrections ===================
            for h in range(H):
                hs_ = slice(h * 128, (h + 1) * 128)
                for l in range(C):
                    csl = slice(l * BLOCK, (l + 1) * BLOCK)
                    sl = kvb_pool.tile([128, 128], BF16, tag="kvb")
                    nc.scalar.activation(out=sl[:, :], in_=S[:, hs_],
                                         func=AF.Copy, scale=float(BD[h] ** l))
                    qdc = qk_pool.tile([128, BLOCK], BF16, tag="qk",
                                       name=f"qdc5_{h}_{l}")
                    nc.sync.dma_start(
                        out=qdc[:, :],
                        in_=bass.AP(tensor=qdec_d, offset=h * BLOCK,
                                    ap=[[0, 128], [1, BLOCK]]))
                    qd_t = qk_pool.tile([128, BLOCK], BF16, tag="qk")
                    nc.vector.tensor_tensor(qd_t[:, :], qT[:, h, csl],
                                            qdc[:, :], ALU.mult)
                    oc = psum_pool.tile([128, BLOCK], F32, tag="ps")
                    nc.tensor.matmul(oc[:, :], sl[:, :], qd_t[:, :],
                                     start=True, stop=True)
                    nc.vector.tensor_tensor(attnT[:, h, csl], attnT[:, h, csl],
                                            oc[:, :], ALU.add)

            if dbg:
                nc.gpsimd.dma_start(out=dbg_d["dbg_qT"].ap(), in_=qT[:, :, :])
                nc.gpsimd.dma_start(out=dbg_d["dbg_kT"].ap(), in_=kT[:, :, :])
                nc.gpsimd.dma_start(out=dbg_d["dbg_v"].ap(), in_=v_sb[:, :, :])
                nc.gpsimd.dma_start(out=dbg_d["dbg_sig"].ap(), in_=sig[:, :, :])
                nc.gpsimd.dma_start(out=dbg_d["dbg_attnT"].ap(), in_=attnT[:, :, :])
                nc.gpsimd.dma_start(out=dbg_d["dbg_U"].ap(), in_=ub[:, :])
                nc.gpsimd.dma_start(out=dbg_d["dbg_S"].ap(), in_=S[:, :])
            # ================= P6: RMS norm + gate ==========================
            rms1s = []
            for ni in range(2):
                nsl = slice(ni * 512, (ni + 1) * 512)
                sp = psum_pool.tile([1, 512], F32, tag="ps", name=f"sp_{ni}")
                for ft in range(H):
                    sq = scratch_pool.tile([128, 512], BF16, tag="sc")
                    nc.scalar.activation(out=sq[:, :], in_=attnT[:, ft, nsl],
                                         func=AF.Square)
                    nc.tensor.matmul(sp[:, :], ones[:, :], sq[:, :],
                                     start=(ft == 0), stop=(ft == H - 1))
                rms1 = scratch_pool.tile([1, 512], F32, tag="sc",
                                         name=f"rms1_{ni}")
                rms1s.append(rms1)
                nc.scalar.activation(out=rms1[:, :], in_=sp[:, :],
                                     func=AF.Sqrt, bias=eps_t[:, :],
                                     scale=float(1.0 / HID))
                nc.vector.reciprocal(out=rms1[:, :], in_=rms1[:, :])
            # broadcast rms over partitions via DRAM round-trip
            for ni in range(2):
                nc.sync.dma_start(out=rms_dram[:, ni * 512:(ni + 1) * 512],
                                  in_=rms1s[ni][:, :])
            nc.gpsimd.dma_start(
                out=rmsb[:, :],
                in_=bass.AP(tensor=rms_dram.tensor_handle
                            if hasattr(rms_dram, "tensor_handle") else rms_dram.tensor,
                            offset=0, ap=[[0, 128], [1, T]]),
            )
            if dbg:
                nc.gpsimd.dma_start(out=dbg_d["dbg_rms"].ap(), in_=rms_dram[:, :])
            for ft in range(H):
                nc.vector.tensor_tensor(attnT[:, ft, :], attnT[:, ft, :],
                                        sig[:, ft, :], ALU.mult)
                nc.vector.tensor_tensor(attnT[:, ft, :], attnT[:, ft, :],
                                        rmsb[:, :], ALU.mult)

            # ================= P7: output projection ========================
            for tq in range(2):
                for njp in range(2):
                    po = [psum_pool.tile([128, 512], F32, tag="ps", name=f"po_{tq}_{njp}_{i}")
                          for i in range(8)]
                    for ft in range(KI):
                        wdt = wvd_pool.tile([128, 1024], BF16, tag="wvd")
                        nc.sync.dma_start(out=wdt[:, :], in_=wd_d[njp, ft])
                        for tt in range(4):
                            t = tq * 4 + tt
                            for njc in range(2):
                                nc.tensor.matmul(
                                    po[tt * 2 + njc][:, :],
                                    attnT[:, ft, t * 128:(t + 1) * 128],
                                    wdt[:, njc * 512:(njc + 1) * 512],
                                    start=(ft == 0), stop=(ft == KI - 1),
                                )
                    for tt in range(4):
                        t = tq * 4 + tt
                        for njc in range(2):
                            ob = scratch_pool.tile([128, 512], F32, tag="sc")
                            nc.vector.tensor_copy(ob[:, :], po[tt * 2 + njc][:, :])
                            nc.sync.dma_start(
                                out=out_d.ap()[t * 128:(t + 1) * 128,
                                               (njp * 2 + njc) * 512:
                                               (njp * 2 + njc + 1) * 512],
                                in_=ob[:, :])

    nc.compile()
    return nc


_CACHE: dict = {}


def _get_program():
    if "nc" not in _CACHE:
        _CACHE["nc"] = build_program()
    return _CACHE["nc"]


def build_in_maps(inputs):
    hs = np.asarray(inputs["hidden_states"], np.float32)
    pos = np.asarray(inputs["positions"]).astype(np.int64)
    Wqkv = np.asarray(inputs["Wqkv"], np.float32)
    Wg = np.asarray(inputs["Wg"], np.float32)
    Wd = np.asarray(inputs["Wd"], np.float32)
    g = np.asarray(inputs["g_norm_weight"], np.float32)

    s = SLOPES.astype(np.float64)
    arr = np.arange(1, BLOCK + 1, dtype=np.float64)
    qdec = (np.exp(-s[:, None] * arr[None, :]) * SC).astype(BF16_NP)
    kdec = np.exp(-s[:, None] * (BLOCK - arr)[None, :]).astype(np.float32)
    idx = arr[:, None] - arr[None, :]
    diag = np.where(idx[None] >= 0, np.exp(-s[:, None, None] * idx[None]), 0.0) * SC
    diagT = np.ascontiguousarray(
        diag.transpose(0, 2, 1).reshape(H, 2, 128, BLOCK)
        .transpose(0, 2, 1, 3)).astype(BF16_NP)

    W_all = np.concatenate([Wqkv[:, :2048], Wqkv[:, 2048:4096], Wg], axis=1)
    wqkg = np.ascontiguousarray(
        W_all.reshape(KI, 128, 48, 128).transpose(2, 1, 0, 3)).astype(BF16_NP)
    wv = np.ascontiguousarray(
        Wqkv[:, 4096:].reshape(KI, 128, 2, 1024).transpose(2, 0, 1, 3)).astype(BF16_NP)
    Wdg = g[:, None] * Wd
    wd = np.ascontiguousarray(
        Wdg.reshape(KI, 128, 2, 1024).transpose(2, 0, 1, 3)).astype(BF16_NP)

    inv_freq = 1.0 / (THETA ** (np.arange(0, D, 2, dtype=np.float64) / D))

    in_maps = []
    for c in range(NCORES):
        rows = slice(c * T, (c + 1) * T)
        hTc = np.ascontiguousarray(
            hs[rows].T.reshape(KI, 128, T).transpose(1, 0, 2)).astype(BF16_NP)
        freqs = pos[rows].astype(np.float64)[:, None] * inv_freq  # [T, 64]
        cos1 = np.cos(freqs).T
        sin1 = np.sin(freqs).T
        cosT = np.ascontiguousarray(np.concatenate([cos1, cos1], 0)).astype(BF16_NP)
        sinT = np.ascontiguousarray(np.concatenate([sin1, sin1], 0)).astype(BF16_NP)
        scanw = np.zeros((NCORES, H), np.float32)
        for cp in range(c):
            scanw[cp] = (BD.astype(np.float64) ** (C * (c - cp - 1))).astype(np.float32)
        in_maps.append({
            "hT": hTc, "wqkg": wqkg, "wv": wv, "wd": wd,
            "cosT": cosT, "sinT": sinT, "qdec": qdec, "kdec": kdec,
            "diagT": diagT, "scanw": scanw,
        })
    return in_maps


def kernel(**inputs):
    nc = _get_program()
    in_maps = build_in_maps(inputs)
    res = run_bass_kernel_spmd(nc, in_maps, core_ids=list(range(NCORES)))
    outs = res.results
    return np.concatenate([np.asarray(outs[c]["out"], np.float32)
                           for c in range(NCORES)], axis=0)
